# revision 16
# baseline (speedup 1.0000x reference)
"""Additive attention kernel for 8 Trainium2 NeuronCores.

Math: scores[b,i,j] = sum_d tanh(q[b,i,d] + k[b,j,d]); out = softmax_j(scores) @ v.

tanh(s) ~= sum_m C[m] sin(W[m] s) (M=8, refit with bf16-exact W[m]/2pi), and
sin(w(q+k)) = sin(wq)cos(wk) + cos(wq)sin(wk) is separable -> scores become a
rank-1024 PE matmul in bf16 (f32 matmul runs as two slow LOW_HIGH passes; bf16
is a single full-rate pass).

Angle path, in turns (t = w x / 2pi), all matmuls bf16 with exact operands:
  t0_psum   = diag(w/2pi) @ (x_hi + x_lo)      (PE; host splits x = hi+lo bf16)
  cos bank += 0.25 (rank-1 ones pass)          (PE; shifts the rounding point)
  n         = (t0 + MAGIC) - MAGIC             (DVE, f32 magic round -> bf16 ints)
  red_psum += (-I) @ n                         (PE; red in [-0.5, 0.5] turns)
  feat      = Sin(2pi * red)  -> bf16          (ScalarE, PSUM -> SBUF, pair-merged)
K-side banks are swapped (cos first) so chunk products pair sin with cos.
Amplitudes: DVE bf16 tensor_scalar with per-partition table. Softmax without
max-subtraction; denominator via a ones-column in V; DVE reciprocal normalizes.

Sharding: B=8 -> one batch per core, no collectives.
"""

import math

import numpy as np
import ml_dtypes

import concourse.bass as bass
import concourse.mybir as mybir
from concourse.bass_utils import run_bass_kernel_spmd

F32 = mybir.dt.float32
BF16 = mybir.dt.bfloat16
AF = mybir.ActivationFunctionType
ALU = mybir.AluOpType

# base fit (amplitudes refit below against bf16-exact frequencies)
W0 = [0.273822509, 0.825679394, 1.38832881, 1.96485759,
      2.55624192, 3.16272728, 3.77941797, 4.47596827]

B, L, D, M = 8, 512, 64, 8
PI = math.pi
TWO_PI = 2.0 * math.pi
MAGIC = 12582912.0  # 1.5 * 2^23

PAIRS = [(0, 1), (2, 3), (4, 5), (6, 7)]


def _bf(x):
    return np.asarray(x).astype(ml_dtypes.bfloat16)


def _fit_consts():
    w2pi = _bf(np.array(W0, np.float32) / TWO_PI).astype(np.float64)
    w_eff = w2pi * TWO_PI
    S = 9.8
    sg = np.linspace(-S, S, 4001)
    wts = np.exp(-(sg**2) / 4) + 0.02
    A = np.sin(np.outer(sg, w_eff)) * np.sqrt(wts)[:, None]
    c, *_ = np.linalg.lstsq(A, np.tanh(sg) * np.sqrt(wts), rcond=None)
    return w2pi.astype(np.float32), c.astype(np.float32)


W2PI, C = _fit_consts()

_CACHE = {}


def _build():
    nc = bass.Bass()
    qhl_ext = nc.declare_dram_parameter("qhl", [128, L], BF16, isOutput=False)
    khl_ext = nc.declare_dram_parameter("khl", [128, L], BF16, isOutput=False)
    vh_ext = nc.declare_dram_parameter("vh", [L, 65], BF16, isOutput=False)
    dg_ext = nc.declare_dram_parameter("dg", [128, 6, 128], BF16, isOutput=False)
    amp_ext = nc.declare_dram_parameter("amp", [128, 4], F32, isOutput=False)
    out_ext = nc.declare_dram_parameter("out", [L, D], F32, isOutput=True)

    from contextlib import ExitStack

    with ExitStack() as ctx:
        e = ctx.enter_context
        QHL = e(nc.sbuf_tensor("QHL", [128, L], BF16))
        KHL = e(nc.sbuf_tensor("KHL", [128, L], BF16))
        DG = e(nc.sbuf_tensor([128, 6, 128], BF16))  # 0-3: diag(w/2pi) double-diag; 4: -I
        AMP = e(nc.sbuf_tensor([128, 4], F32))
        VH = e(nc.sbuf_tensor([128, 4, 65], BF16))
        NS = e(nc.sbuf_tensor([128, 2, 4, L], BF16))
        NC_ = e(nc.sbuf_tensor("NCT", [128, 2, 4, L], BF16))
        AC = e(nc.sbuf_tensor([128, 2, 4, L], F32))
        FQRAW = e(nc.sbuf_tensor([128, 4, 2, L], BF16))
        FQS = e(nc.sbuf_tensor([128, 4, 2, L], BF16))
        FK = e(nc.sbuf_tensor([128, 4, 2, L], BF16))
        EXPT = e(nc.sbuf_tensor([128, 4, L], BF16))
        RCP = e(nc.sbuf_tensor([128, 4], F32))
        OUT = e(nc.sbuf_tensor([128, 4, D], F32))
        PSUMS = e(nc.psum_tensor([128, 4 * L], F32))
        PSUMR = e(nc.psum_tensor([128, 4, L], F32))
        s_in = e(nc.semaphore("s_in"))
        s_vh = e(nc.semaphore("s_vh"))
        s_t0 = e(nc.semaphore("s_t0"))
        s_n = e(nc.semaphore("s_n"))
        s_red = e(nc.semaphore("s_red"))
        s_act = e(nc.semaphore("s_act"))
        s_amp = e(nc.semaphore("s_amp"))
        s_scores = e(nc.semaphore("s_scores"))
        s_exp = e(nc.semaphore("s_exp"))
        s_av = e(nc.semaphore("s_av"))
        s_rcp = e(nc.semaphore("s_rcp"))
        s_norm = e(nc.semaphore("s_norm"))
        block = e(nc.Block())

        XHL = [QHL, KHL]

        # units: g = 2*pair + side (Q first). Banks: rA=(2g)%4, rB=rA+1.
        # Q: A=sin, B=cos; K: A=cos, B=sin (so FK comes out [cos|sin]).
        def banks(g):
            rA = (2 * g) % 4
            return rA, rA + 1

        @block.sync
        def _(sync):
            sync.dma_start(out=QHL[:], in_=qhl_ext[:]).then_inc(s_in, 16)
            sync.dma_start(out=DG[:], in_=dg_ext[:]).then_inc(s_in, 16)
            sync.dma_start(out=AMP[:], in_=amp_ext[:]).then_inc(s_in, 16)
            sync.wait_ge(s_norm, 4)
            sync.dma_start(
                out=out_ext.rearrange("(g p) c -> p g c", p=128), in_=OUT[:]
            ).then_inc(s_in, 16)

        @block.gpsimd
        def _(gpsimd):
            gpsimd.dma_start(out=KHL[:], in_=khl_ext[:]).then_inc(s_in, 16)
            gpsimd.dma_start(
                out=VH[:], in_=vh_ext.rearrange("(g p) c -> p g c", p=128)
            ).then_inc(s_vh, 16)

        @block.vector
        def _(vector):
            for g in range(8):
                j, s = g // 2, g % 2
                rA, rB = banks(g)
                sin_bank, cos_bank = (rA, rB) if s == 0 else (rB, rA)
                vector.wait_ge(s_t0, g + 1)
                vector.tensor_scalar(
                    AC[:, s, j, :], PSUMR[:, sin_bank, :], 0.25, MAGIC,
                    ALU.add, ALU.add,
                )
                vector.tensor_scalar(
                    NS[:, s, j, :], PSUMR[:, sin_bank, :], MAGIC, -MAGIC,
                    ALU.add, ALU.add,
                ).then_inc(s_n, 1)
                vector.tensor_scalar(
                    NC_[:, s, j, :], AC[:, s, j, :], -MAGIC, -0.25,
                    ALU.add, ALU.add,
                ).then_inc(s_n, 1)
                if s == 1:
                    vector.wait_ge(s_act, 3 * j + 1)
                    vector.tensor_scalar_mul(
                        FQS[:, j], FQRAW[:, j], AMP[:, j : j + 1]
                    ).then_inc(s_amp, 1)
            for ib in range(4):
                vector.wait_ge(s_av, ib + 1)
                vector.reciprocal(RCP[:, ib : ib + 1], PSUMR[:, ib, 64:65]).then_inc(
                    s_rcp, 1
                )

        @block.scalar
        def _(scalar):
            for g in range(8):
                j, s = g // 2, g % 2
                rA, _ = banks(g)
                scalar.wait_ge(s_red, 2 * g + 2)
                if s == 0:
                    scalar.activation(
                        FQRAW[:, j], PSUMR[:, rA : rA + 2, :], AF.Sin,
                        scale=TWO_PI,
                    ).then_inc(s_act, 1)
                else:
                    scalar.activation(
                        FK[:, j, 0], PSUMR[:, rA, :], AF.Sin, scale=TWO_PI
                    ).then_inc(s_act, 1)
                    scalar.activation(
                        FK[:, j, 1], PSUMR[:, rA + 1, :], AF.Sin, scale=TWO_PI
                    ).then_inc(s_act, 1)
            for jb in range(4):
                scalar.wait_ge(s_scores, jb + 1)
                scalar.activation(
                    EXPT[:, jb], PSUMS[:, jb * L : (jb + 1) * L], AF.Exp
                ).then_inc(s_exp, 1)
            for ib in range(4):
                scalar.wait_ge(s_rcp, ib + 1)
                scalar.activation(
                    OUT[:, ib, :], PSUMR[:, ib, 0:D], AF.Identity,
                    scale=RCP[:, ib : ib + 1],
                ).then_inc(s_norm, 1)

        @block.tensor
        def _(tensor):
            def red_t0(g):
                j, s = g // 2, g % 2
                rA, rB = banks(g)
                sin_bank, cos_bank = (rA, rB) if s == 0 else (rB, rA)
                tensor.wait_ge(s_in, 64)
                if g >= 2:
                    # banks freed by the acts of unit g-2 (split-act numbering)
                    need = {2: 1, 3: 3, 4: 4, 5: 6, 6: 7, 7: 9}[g]
                    tensor.wait_ge(s_act, need)
                tensor.matmul(PSUMR[:, sin_bank, :], DG[:, j, :], XHL[s][:],
                              start=True, stop=False).then_inc(s_t0, 1)
                tensor.matmul(PSUMR[:, cos_bank, :], DG[:, j, :], XHL[s][:],
                              start=True, stop=False)

            def red_fin(g):
                j, s = g // 2, g % 2
                rA, rB = banks(g)
                sin_bank, cos_bank = (rA, rB) if s == 0 else (rB, rA)
                tensor.wait_ge(s_n, 2 * g + 1)
                tensor.matmul(PSUMR[:, sin_bank, :], DG[:, 4, :], NS[:, s, j, :],
                              start=False, stop=True).then_inc(s_red, 1)
                tensor.wait_ge(s_n, 2 * g + 2)
                tensor.matmul(PSUMR[:, cos_bank, :], DG[:, 4, :], NC_[:, s, j, :],
                              start=False, stop=True).then_inc(s_red, 1)

            def scores(j):
                tensor.wait_ge(s_amp, j + 1)
                for t in range(2):
                    tensor.wait_ge(s_act, 3 * j + 2 + t)
                    for jb in range(4):
                        mm = tensor.matmul(
                            PSUMS[:, jb * L : (jb + 1) * L],
                            FK[:, j, t, jb * 128 : (jb + 1) * 128],
                            FQS[:, j, t, :],
                            start=(j == 0 and t == 0),
                            stop=(j == 3 and t == 1),
                        )
                        if j == 3 and t == 1:
                            mm.then_inc(s_scores, 1)
                return mm

            # two-deep pipeline: t0(g), t0(g+1), fin(g), fin(g+1), scores...
            red_t0(0)
            red_t0(1)
            red_fin(0)
            red_fin(1)
            for j in range(4):
                for g in (2 * j + 2, 2 * j + 3):
                    if g < 8:
                        red_t0(g)
                for g in (2 * j + 2, 2 * j + 3):
                    if g < 8:
                        red_fin(g)
                mm = scores(j)

            tensor.wait_ge(s_vh, 16)
            for jb in range(4):
                tensor.wait_ge(s_exp, jb + 1)
                for ib in range(4):
                    mm = tensor.matmul(
                        PSUMR[:, ib, 0:65],
                        EXPT[:, jb, ib * 128 : (ib + 1) * 128],
                        VH[:, jb, :],
                        start=(jb == 0),
                        stop=(jb == 3),
                    )
                    if jb == 3:
                        mm.then_inc(s_av, 1)

    return nc


def _get_nc():
    if "nc" not in _CACHE:
        _CACHE["nc"] = _build()
    return _CACHE["nc"]


def _make_consts():
    dg = np.zeros((128, 6, 128), np.float32)
    amp = np.zeros((128, 4), np.float32)
    for j, (a, b) in enumerate(PAIRS):
        for p in range(64):
            # out col p (freq a, d=p) reads xh row p and xl row 64+p
            dg[p, j, p] = W2PI[a]
            dg[64 + p, j, p] = W2PI[a]
            # out col 64+p (freq b, d=p) reads the same rows
            dg[p, j, 64 + p] = W2PI[b]
            dg[64 + p, j, 64 + p] = W2PI[b]
        amp[0:64, j] = C[a]
        amp[64:128, j] = C[b]
    for p in range(128):
        dg[p, 4, p] = -1.0
    return _bf(dg), amp


def _make_in_maps(q, k, v):
    dg, amp = _make_consts()
    in_maps = []
    for b in range(B):
        def hilo(x):
            xt = np.ascontiguousarray(x.T.astype(np.float32))      # [64, 512]
            h = _bf(xt)
            lo = _bf(xt - h.astype(np.float32))
            return np.concatenate([h, lo], axis=0)                  # [128, 512]

        qhl = hilo(q[b])
        khl = hilo(k[b])
        vh = _bf(np.concatenate(
            [v[b].astype(np.float32), np.ones((L, 1), np.float32)], axis=1
        ))
        in_maps.append({"qhl": qhl, "khl": khl,
                        "vh": vh, "dg": dg, "amp": amp})
    return in_maps


def _run(in_maps, **kw):
    nc = _get_nc()
    return run_bass_kernel_spmd(nc, in_maps, core_ids=list(range(8)), **kw)


def kernel(q: np.ndarray, k: np.ndarray, v: np.ndarray) -> np.ndarray:
    res = _run(_make_in_maps(q, k, v))
    out = np.stack([res.results[b]["out"] for b in range(B)]).astype(np.float32)
    return out
